# revision 10
# baseline (speedup 1.0000x reference)
"""Trainium2 Bass kernel for nn_Aggregator (GNN relational message passing).

Computes: out[h] = sum_{e: head_e==h} all_emb[tail_e] * weight[type_e] * aug_e

Strategy (8 NeuronCores, SPMD):
  - Shard output nodes (head ranges of 12500) across the 8 cores; each core
    gets exactly the edges whose head falls in its range (host bins them).
    No collective needed - host concatenates the 8 output shards.
  - all_emb is cast to fp16 on host and viewed as 50000 node-PAIR tokens of
    256B each (dma_gather requires 256B-multiple elements).  Pair indices
    fit in 2 int16 windows of 25000, halving the (tile, window) bucket
    count vs fp32 rows and cutting slot padding from 34% to 17%.  The
    dma_gather descriptor generation on the GpSimd Q7 cores (~8ns/row per
    queue-pair, 4 queue-pairs in parallel) is the kernel's critical path,
    so padded slots directly cost wall clock.
  - Host sorts edges by (head tile of 128, tail window); each (tile,window)
    bucket is padded to capw chunks of 128 edges so the schedule is static.
  - Per 128-edge chunk (each edge slot carries a 128-wide token = 2 nodes;
    a host-built half-mask in the weight table selects the right node):
      S    = is_equal(iota128, head) via DVE tensor_scalar (4x mode) [e,128]
      Wsel = oh^T @ w2 on PE (2 chunks stacked: 96 rows, 256 out cols)
             -> aug_e*weight[type_e] in the edge's half, 0 in the other
      wsel16 = ACT copy PSUM->SBUF fp16
      V    = G * wsel16 on DVE (fp16, 2x mode)                       [e,128]
      acc(psum f32) += S^T @ V on PE  (accumulated per node tile)
    Tile flush: out[:,c] = acc[:,c] + acc[:,64+c] (DVE) into a per-group
    staging buffer; one output DMA per 7-tile group.
"""

import os

import numpy as np

import concourse.bacc as bacc
import concourse.tile as tile
from concourse import bass, mybir
from concourse.bass_utils import run_bass_kernel_spmd

P = 128
C = 64  # channels
R = 24  # relations
TOK = 128  # fp16 token width (2 nodes x 64 channels)
N_NODES = 100000
NPAIR = N_NODES // 2
N_CORES = 8
NPC = N_NODES // N_CORES  # 12500 nodes per core
TILES = (NPC + P - 1) // P  # 98 output tiles per core
WINS = 2
WSZ = 25000  # window size in pair units (int16 gather index reach)
GROUP_T = 7  # tiles per gather group (98 = 14 * 7)
NGRP = TILES // GROUP_T

_NC_CACHE = {}


def _ap3(ap, offset_elems, mid_step, mid_n, inner_n):
    """[128, mid_n, inner_n] view of a 2D SBUF tile AP with custom strides."""
    return bass.AP(
        ap.tensor,
        ap.offset + offset_elems,
        [list(ap.ap[0]), [mid_step, mid_n], [1, inner_n]],
    )


def _build(capw: int):
    """Build the Bass module for per-(tile,window) chunk capacity capw."""
    cap = WINS * capw  # chunks per tile
    nchunk = TILES * cap
    gc = GROUP_T * cap  # chunks per group
    gcw = GROUP_T * capw  # chunks per (group, window) gather call
    npair_oh = cap // 2  # stacked wsel matmuls per tile

    nc = bacc.Bacc("TRN2", target_bir_lowering=False, num_swdge_queues=4)

    f32 = mybir.dt.float32
    f16 = mybir.dt.float16
    bf16 = mybir.dt.bfloat16
    i16 = mybir.dt.int16

    emb_d = nc.dram_tensor("emb16", [NPAIR, TOK], f16, kind="ExternalInput")
    idx_d = nc.dram_tensor("idx16", [P, nchunk * 8], i16, kind="ExternalInput")
    head_d = nc.dram_tensor("head_local", [P, nchunk], f32,
                            kind="ExternalInput")
    oh_d = nc.dram_tensor("oh", [P, (nchunk // 2) * P], bf16,
                          kind="ExternalInput")
    w2_d = nc.dram_tensor("w2", [P, 2 * TOK], bf16, kind="ExternalInput")
    iota_d = nc.dram_tensor("iota", [P, P], f16, kind="ExternalInput")
    out_d = nc.dram_tensor("out", [TILES * P, C], f32, kind="ExternalOutput")

    with tile.TileContext(nc) as tc:
        with (
            tc.tile_pool(name="res", bufs=1) as res,
            tc.tile_pool(name="gp", bufs=3) as gp,
            tc.tile_pool(name="ohp", bufs=2) as ohp,
            tc.tile_pool(name="sp", bufs=2) as sp,
            tc.tile_pool(name="vp", bufs=2) as vp,
            tc.tile_pool(name="wp", bufs=2) as wp,
            tc.tile_pool(name="op", bufs=2) as op,
            tc.tile_pool(name="fp", bufs=2) as fp,
            tc.tile_pool(name="pp", bufs=2, space="PSUM") as pp,
        ):
            idx_t = res.tile([P, nchunk * 8], i16)
            head_t = res.tile([P, nchunk], f32)
            iota_t = res.tile([P, P], f16)
            w2_t = res.tile([P, 2 * TOK], bf16)

            nc.sync.dma_start(idx_t[:], idx_d[:])
            nc.sync.dma_start(head_t[:], head_d[:])
            nc.sync.dma_start(iota_t[:], iota_d[:])
            nc.sync.dma_start(w2_t[:], w2_d[:])

            for g in range(NGRP):
                # gather the whole group, one call per source window
                g_t = gp.tile([P, gc * TOK], f16, tag="g")
                for w in range(WINS):
                    nidx = gcw * P
                    call = g * WINS + w
                    s0 = call * nidx  # first flat gather slot
                    nc.gpsimd.dma_gather(
                        out_ap=g_t[
                            :, w * gcw * TOK : (w + 1) * gcw * TOK
                        ].rearrange("p (j t) -> p j t", t=TOK),
                        in_ap=emb_d[w * WSZ : (w + 1) * WSZ, :],
                        idxs_ap=idx_t[:, s0 // 16 : s0 // 16 + nidx // 16],
                        num_idxs=nidx,
                        num_idxs_reg=nidx,
                        elem_size=TOK,
                        single_packet=False,
                        queue_num=call % 4,
                    )

                # onehot slab for the group's chunks
                oh_t = ohp.tile([P, (gc // 2) * P], bf16, tag="oh")
                nc.sync.dma_start(
                    oh_t[:], oh_d[:, (g * gc // 2) * P : ((g + 1) * gc // 2) * P]
                )

                ostage_t = op.tile([P, GROUP_T * C], f32, tag="ost")

                for tloc in range(GROUP_T):
                    t = g * GROUP_T + tloc
                    j0 = t * cap  # first chunk of the tile

                    # head one-hots: per chunk via tensor_scalar (4x DVE)
                    s_t = sp.tile([P, cap * P], f16, tag="s")
                    for q in range(cap):
                        nc.vector.tensor_scalar(
                            out=s_t[:, q * P : (q + 1) * P],
                            in0=iota_t[:],
                            scalar1=head_t[:, j0 + q : j0 + q + 1],
                            scalar2=None,
                            op0=mybir.AluOpType.is_equal,
                        )

                    # one PSUM tile per output tile: wsel (cap*128 fp32)
                    # followed by the 128-col accumulator (fits 4 banks)
                    ps_t = pp.tile([P, cap * TOK + P], f32, tag="ps")

                    # Wsel: one stacked matmul per 2 chunks
                    for b in range(npair_oh):
                        gcol = (j0 // 2 + b) * P - (g * gc // 2) * P
                        nc.tensor.matmul(
                            out=ps_t[:, b * 2 * TOK : (b + 1) * 2 * TOK],
                            lhsT=oh_t[:, gcol : gcol + P],
                            rhs=w2_t[:],
                            start=True,
                            stop=True,
                        )

                    # PSUM fp32 -> SBUF fp16 on ACT
                    wsel16_t = wp.tile([P, cap * TOK], f16, tag="w16")
                    nc.scalar.copy(out=wsel16_t[:], in_=ps_t[:, 0 : cap * TOK])

                    # V = G * wsel16 for the whole tile in one DVE op.
                    v_t = vp.tile([P, cap * TOK], f16, tag="v")
                    g_view = _ap3(
                        g_t[:],
                        tloc * capw * TOK,
                        gcw * TOK,
                        WINS,
                        capw * TOK,
                    )
                    nc.vector.tensor_tensor(
                        out=v_t[:].rearrange("p (w x) -> p w x", w=WINS),
                        in0=g_view,
                        in1=wsel16_t[:].rearrange("p (w x) -> p w x", w=WINS),
                        op=mybir.AluOpType.mult,
                    )

                    # scatter-accumulate the tile's chunks into PSUM
                    a0 = cap * TOK
                    for q in range(cap):
                        nc.tensor.matmul(
                            out=ps_t[:, a0 : a0 + P],
                            lhsT=s_t[:, q * P : (q + 1) * P],
                            rhs=v_t[:, q * TOK : (q + 1) * TOK],
                            start=(q == 0),
                            stop=(q == cap - 1),
                        )

                    # fold the two node-halves and stage (DVE can read at
                    # most one PSUM operand: ACT stages the high half first)
                    fold_t = fp.tile([P, C], f32, tag="fold")
                    nc.scalar.copy(out=fold_t[:], in_=ps_t[:, a0 + C : a0 + 2 * C])
                    nc.vector.tensor_tensor(
                        out=ostage_t[:, tloc * C : (tloc + 1) * C],
                        in0=ps_t[:, a0 : a0 + C],
                        in1=fold_t[:],
                        op=mybir.AluOpType.add,
                    )

                nc.sync.dma_start(
                    out_d[g * GROUP_T * P : (g + 1) * GROUP_T * P, :].rearrange(
                        "(t p) c -> p t c", p=P
                    ),
                    ostage_t[:].rearrange("p (t c) -> p t c", c=C),
                )

    nc.finalize()
    return nc


def _prep(all_emb, edge_index, edge_type, weight, aug_edge_weight):
    """Host-side binning/padding. Returns (capw, in_maps)."""
    head = np.asarray(edge_index[0], dtype=np.int64)
    tail = np.asarray(edge_index[1], dtype=np.int64)
    etype = np.asarray(edge_type, dtype=np.int64)
    aug = np.asarray(aug_edge_weight, dtype=np.float32).reshape(-1)
    emb16 = np.ascontiguousarray(
        np.asarray(all_emb, dtype=np.float16).reshape(NPAIR, TOK)
    )
    w = np.asarray(weight, dtype=np.float32)

    order = np.argsort(head, kind="stable")
    h_s = head[order]
    bounds = np.searchsorted(h_s, np.arange(N_CORES + 1) * NPC)

    capw = 1
    per_core = []
    for c_i in range(N_CORES):
        e_idx = order[bounds[c_i] : bounds[c_i + 1]]
        h_loc = h_s[bounds[c_i] : bounds[c_i + 1]] - c_i * NPC
        t_loc = tail[e_idx]
        win = (t_loc >> 1) // WSZ
        tw = (h_loc >> 7) * WINS + win  # (tile, window) bucket
        cnt = np.bincount(tw, minlength=TILES * WINS)
        capw = max(capw, int(-(-cnt.max() // P)))
        per_core.append((e_idx, h_loc, t_loc, tw, cnt))

    cap = WINS * capw
    nchunk = TILES * cap
    gcw = GROUP_T * capw

    import ml_dtypes

    iota = np.tile(np.arange(P, dtype=np.float16), (P, 1))
    # w2[48s + 24h + r, s*128 + 64h + c] = w[r, c]
    w2 = np.zeros((P, 2 * TOK), dtype=np.float32)
    for s in range(2):
        for h in range(2):
            w2[48 * s + 24 * h : 48 * s + 24 * h + R,
               s * TOK + C * h : s * TOK + C * (h + 1)] = w
    w2 = w2.astype(ml_dtypes.bfloat16)

    in_maps = []
    for c_i in range(N_CORES):
        e_idx, h_loc, t_loc, tw, cnt = per_core[c_i]
        o2 = np.argsort(tw, kind="stable")  # group edges by (tile, window)
        e_idx, h_loc, t_loc, tw = e_idx[o2], h_loc[o2], t_loc[o2], tw[o2]
        starts = np.cumsum(cnt) - cnt
        pos = np.arange(len(e_idx)) - starts[tw]

        tile_id = tw // WINS
        win = tw % WINS
        k = pos >> 7
        p = pos & (P - 1)

        pair_local = (t_loc >> 1) - win * WSZ
        half = t_loc & 1

        # processing chunk id within tile / globally
        j = win * capw + k
        jg = tile_id * cap + j
        # flat gather slot: call (g, w), then (tloc, k, p)
        grp = tile_id // GROUP_T
        tloc = tile_id % GROUP_T
        call = grp * WINS + win
        gi = call * (gcw * P) + (tloc * capw + k) * P + p

        idx16 = np.zeros((P, nchunk * 8), np.int16)
        val16 = pair_local.astype(np.int16)
        rows = (gi % 16).astype(np.int64)
        cols = (gi // 16).astype(np.int64)
        for rep in range(8):
            idx16[rep * 16 + rows, cols] = val16

        head_a = np.zeros((P, nchunk), dtype=np.float32)
        head_a[p, jg] = (h_loc - (tile_id << 7)).astype(np.float32)

        oh = np.zeros((P, (nchunk // 2) * P), dtype=ml_dtypes.bfloat16)
        q_r = 48 * (j & 1) + 24 * half + etype[e_idx]
        col = (jg >> 1) * P + p
        oh[q_r, col] = aug[e_idx].astype(ml_dtypes.bfloat16)

        in_maps.append(
            {
                "emb16": emb16,
                "idx16": idx16,
                "head_local": head_a,
                "oh": oh,
                "w2": w2,
                "iota": iota,
            }
        )
    return capw, in_maps


def kernel(all_emb, edge_index, edge_type, weight, aug_edge_weight):
    capw, in_maps = _prep(all_emb, edge_index, edge_type, weight,
                          aug_edge_weight)
    if capw not in _NC_CACHE:
        _NC_CACHE[capw] = _build(capw)
    nc = _NC_CACHE[capw]

    trace = bool(int(os.environ.get("KERNEL_TRACE", "0")))
    res = run_bass_kernel_spmd(
        nc,
        in_maps,
        core_ids=list(range(N_CORES)),
        trace=trace,
    )
    kernel._last_results = res
    out = np.concatenate(
        [res.results[c_i]["out"][:NPC] for c_i in range(N_CORES)], axis=0
    )
    return out


# revision 11
# speedup vs baseline: 1.5079x; 1.5079x over previous
"""Trainium2 Bass kernel for nn_Aggregator (GNN relational message passing).

Computes: out[h] = sum_{e: head_e==h} all_emb[tail_e] * weight[type_e] * aug_e

Strategy (8 NeuronCores, SPMD):
  - Shard output nodes (head ranges of 12500) across the 8 cores; each core
    gets exactly the edges whose head falls in its range (host bins them).
    No collective needed - host concatenates the 8 output shards.
  - all_emb is cast to fp16 on host and viewed as 50000 node-PAIR tokens of
    256B each (dma_gather requires 256B-multiple elements).  Pair indices
    fit in 2 int16 windows of 25000, cutting (tile, window) bucket count
    and slot padding (17% vs 34% for fp32 rows).  dma_gather descriptor
    generation on the GpSimd Q7 cores (~8ns/row per queue-pair, 4 pairs
    concurrent when calls rotate queue_num) is the critical path, so each
    group's gather is issued as 4 calls on 4 queues.
  - Host sorts edges by (head tile of 128, tail window); each (tile,window)
    bucket is padded to capw chunks of 128 edges so the schedule is static.
  - Per 128-edge chunk (each edge slot carries a 128-wide token = 2 nodes;
    a host-built half-mask in the weight table selects the right node):
      S    = host-built fp8 one-hot of head (streamed per group, 0/1 exact)
      Wsel = oh^T @ w2 on PE (2 chunks stacked: 96 rows, 256 out cols)
             -> aug_e*weight[type_e] in the edge's half, 0 in the other
      wsel16 = ACT copy PSUM->SBUF fp16
      V    = G * wsel16 on DVE (fp16, 2x mode)                       [e,128]
      acc(psum f32) += S^T @ V on PE (fp8 x fp16, per node tile)
    Tile flush: out[:,c] = acc[:,c] + acc[:,64+c] (ACT stages the high
    half, DVE adds) into a per-group staging buffer; one output DMA per
    7-tile group.
"""

import os

import numpy as np

import concourse.bacc as bacc
import concourse.tile as tile
from concourse import bass, mybir
from concourse.bass_utils import run_bass_kernel_spmd

P = 128
C = 64  # channels
R = 24  # relations
TOK = 128  # fp16 token width (2 nodes x 64 channels)
N_NODES = 100000
NPAIR = N_NODES // 2
N_CORES = 8
NPC = N_NODES // N_CORES  # 12500 nodes per core
TILES = (NPC + P - 1) // P  # 98 output tiles per core
WINS = 2
WSZ = 25000  # window size in pair units (int16 gather index reach)
GROUP_T = 7  # tiles per gather group (98 = 14 * 7)
NGRP = TILES // GROUP_T

_NC_CACHE = {}


def _ap3(ap, offset_elems, mid_step, mid_n, inner_n):
    """[128, mid_n, inner_n] view of a 2D SBUF tile AP with custom strides."""
    return bass.AP(
        ap.tensor,
        ap.offset + offset_elems,
        [list(ap.ap[0]), [mid_step, mid_n], [1, inner_n]],
    )


def _build(capw: int):
    """Build the Bass module for per-(tile,window) chunk capacity capw."""
    cap = WINS * capw  # chunks per tile
    nchunk = TILES * cap
    gc = GROUP_T * cap  # chunks per group
    gcw = GROUP_T * capw  # chunks per (group, window)
    hcw = gcw // 2  # first-half chunks of a (group, window) gather
    npair_oh = cap // 2  # stacked wsel matmuls per tile

    nc = bacc.Bacc("TRN2", target_bir_lowering=False, num_swdge_queues=4)

    f32 = mybir.dt.float32
    f16 = mybir.dt.float16
    bf16 = mybir.dt.bfloat16
    f8 = mybir.dt.float8e4
    i16 = mybir.dt.int16

    emb_d = nc.dram_tensor("emb16", [NPAIR, TOK], f16, kind="ExternalInput")
    idx_d = nc.dram_tensor("idx16", [P, nchunk * 8], i16, kind="ExternalInput")
    s8_d = nc.dram_tensor("s8", [P, nchunk * P], f8, kind="ExternalInput")
    oh_d = nc.dram_tensor("oh", [P, (nchunk // 2) * P], bf16,
                          kind="ExternalInput")
    w2_d = nc.dram_tensor("w2", [P, 2 * TOK], bf16, kind="ExternalInput")
    out_d = nc.dram_tensor("out", [TILES * P, C], f32, kind="ExternalOutput")

    with tile.TileContext(nc) as tc:
        with (
            tc.tile_pool(name="res", bufs=1) as res,
            tc.tile_pool(name="gp", bufs=3) as gp,
            tc.tile_pool(name="ohp", bufs=2) as ohp,
            tc.tile_pool(name="sp", bufs=2) as sp,
            tc.tile_pool(name="vp", bufs=2) as vp,
            tc.tile_pool(name="wp", bufs=2) as wp,
            tc.tile_pool(name="op", bufs=2) as op,
            tc.tile_pool(name="fp", bufs=2) as fp,
            tc.tile_pool(name="pp", bufs=2, space="PSUM") as pp,
        ):
            idx_t = res.tile([P, nchunk * 8], i16)
            w2_t = res.tile([P, 2 * TOK], bf16)

            nc.sync.dma_start(idx_t[:], idx_d[:])
            nc.sync.dma_start(w2_t[:], w2_d[:])

            ncall = 0
            for g in range(NGRP):
                # gather the whole group; split each window into two calls
                # so all 4 SWDGE queue pairs stay busy
                g_t = gp.tile([P, gc * TOK], f16, tag="g")
                for w in range(WINS):
                    for h0, hn in ((0, hcw), (hcw, gcw - hcw)):
                        nidx = hn * P
                        s0 = (g * WINS + w) * gcw * P + h0 * P
                        nc.gpsimd.dma_gather(
                            out_ap=g_t[
                                :,
                                (w * gcw + h0) * TOK : (w * gcw + h0 + hn)
                                * TOK,
                            ].rearrange("p (j t) -> p j t", t=TOK),
                            in_ap=emb_d[w * WSZ : (w + 1) * WSZ, :],
                            idxs_ap=idx_t[
                                :, s0 // 16 : s0 // 16 + nidx // 16
                            ],
                            num_idxs=nidx,
                            num_idxs_reg=nidx,
                            elem_size=TOK,
                            single_packet=False,
                            queue_num=ncall % 4,
                        )
                        ncall += 1

                # one-hot S and aug*weight one-hot slabs for the group
                s8_t = sp.tile([P, gc * P], f8, tag="s8")
                nc.sync.dma_start(
                    s8_t[:], s8_d[:, g * gc * P : (g + 1) * gc * P]
                )
                oh_t = ohp.tile([P, (gc // 2) * P], bf16, tag="oh")
                nc.sync.dma_start(
                    oh_t[:], oh_d[:, (g * gc // 2) * P : ((g + 1) * gc // 2) * P]
                )

                ostage_t = op.tile([P, GROUP_T * C], f32, tag="ost")

                for tloc in range(GROUP_T):
                    j0 = tloc * cap  # first chunk of the tile within group

                    # one PSUM tile per output tile: wsel (cap*128 fp32)
                    # followed by the 128-col accumulator (fits 4 banks)
                    ps_t = pp.tile([P, cap * TOK + P], f32, tag="ps")

                    # Wsel: one stacked matmul per 2 chunks
                    for b in range(npair_oh):
                        gcol = (j0 // 2 + b) * P
                        nc.tensor.matmul(
                            out=ps_t[:, b * 2 * TOK : (b + 1) * 2 * TOK],
                            lhsT=oh_t[:, gcol : gcol + P],
                            rhs=w2_t[:],
                            start=True,
                            stop=True,
                        )

                    # PSUM fp32 -> SBUF fp16 on ACT
                    wsel16_t = wp.tile([P, cap * TOK], f16, tag="w16")
                    nc.scalar.copy(out=wsel16_t[:], in_=ps_t[:, 0 : cap * TOK])

                    # V = G * wsel16 for the whole tile in one DVE op.
                    v_t = vp.tile([P, cap * TOK], f16, tag="v")
                    g_view = _ap3(
                        g_t[:],
                        tloc * capw * TOK,
                        gcw * TOK,
                        WINS,
                        capw * TOK,
                    )
                    nc.vector.tensor_tensor(
                        out=v_t[:].rearrange("p (w x) -> p w x", w=WINS),
                        in0=g_view,
                        in1=wsel16_t[:].rearrange("p (w x) -> p w x", w=WINS),
                        op=mybir.AluOpType.mult,
                    )

                    # scatter-accumulate the tile's chunks into PSUM
                    a0 = cap * TOK
                    for q in range(cap):
                        nc.tensor.matmul(
                            out=ps_t[:, a0 : a0 + P],
                            lhsT=s8_t[:, (j0 + q) * P : (j0 + q + 1) * P],
                            rhs=v_t[:, q * TOK : (q + 1) * TOK],
                            start=(q == 0),
                            stop=(q == cap - 1),
                        )

                    # fold the two node-halves and stage (DVE can read at
                    # most one PSUM operand: ACT stages the high half first)
                    fold_t = fp.tile([P, C], f32, tag="fold")
                    nc.scalar.copy(out=fold_t[:], in_=ps_t[:, a0 + C : a0 + 2 * C])
                    nc.vector.tensor_tensor(
                        out=ostage_t[:, tloc * C : (tloc + 1) * C],
                        in0=ps_t[:, a0 : a0 + C],
                        in1=fold_t[:],
                        op=mybir.AluOpType.add,
                    )

                nc.sync.dma_start(
                    out_d[g * GROUP_T * P : (g + 1) * GROUP_T * P, :].rearrange(
                        "(t p) c -> p t c", p=P
                    ),
                    ostage_t[:].rearrange("p (t c) -> p t c", c=C),
                )

    nc.finalize()
    return nc


def _prep(all_emb, edge_index, edge_type, weight, aug_edge_weight):
    """Host-side binning/padding. Returns (capw, in_maps)."""
    import ml_dtypes

    f8np = mybir.dt.np(mybir.dt.float8e4)
    bf16np = ml_dtypes.bfloat16

    head = np.asarray(edge_index[0], dtype=np.int64)
    tail = np.asarray(edge_index[1], dtype=np.int64)
    etype = np.asarray(edge_type, dtype=np.int64)
    aug = np.asarray(aug_edge_weight, dtype=np.float32).reshape(-1)
    emb16 = np.ascontiguousarray(
        np.asarray(all_emb, dtype=np.float16).reshape(NPAIR, TOK)
    )
    w = np.asarray(weight, dtype=np.float32)

    order = np.argsort(head, kind="stable")
    h_s = head[order]
    bounds = np.searchsorted(h_s, np.arange(N_CORES + 1) * NPC)

    capw = 1
    per_core = []
    for c_i in range(N_CORES):
        e_idx = order[bounds[c_i] : bounds[c_i + 1]]
        h_loc = h_s[bounds[c_i] : bounds[c_i + 1]] - c_i * NPC
        t_loc = tail[e_idx]
        win = (t_loc >> 1) // WSZ
        tw = (h_loc >> 7) * WINS + win  # (tile, window) bucket
        cnt = np.bincount(tw, minlength=TILES * WINS)
        capw = max(capw, int(-(-cnt.max() // P)))
        per_core.append((e_idx, h_loc, t_loc, tw, cnt))

    cap = WINS * capw
    nchunk = TILES * cap
    gcw = GROUP_T * capw

    # w2[48s + 24h + r, s*128 + 64h + c] = w[r, c]
    w2 = np.zeros((P, 2 * TOK), dtype=np.float32)
    for s in range(2):
        for h in range(2):
            w2[48 * s + 24 * h : 48 * s + 24 * h + R,
               s * TOK + C * h : s * TOK + C * (h + 1)] = w
    w2 = w2.astype(bf16np)

    in_maps = []
    for c_i in range(N_CORES):
        e_idx, h_loc, t_loc, tw, cnt = per_core[c_i]
        o2 = np.argsort(tw, kind="stable")  # group edges by (tile, window)
        e_idx, h_loc, t_loc, tw = e_idx[o2], h_loc[o2], t_loc[o2], tw[o2]
        starts = np.cumsum(cnt) - cnt
        pos = np.arange(len(e_idx)) - starts[tw]

        tile_id = tw // WINS
        win = tw % WINS
        k = pos >> 7
        p = pos & (P - 1)

        pair_local = (t_loc >> 1) - win * WSZ
        half = t_loc & 1

        # processing chunk id within tile / globally
        j = win * capw + k
        jg = tile_id * cap + j
        # flat gather slot: (group, window) major, then (tloc, k, p)
        grp = tile_id // GROUP_T
        tloc = tile_id % GROUP_T
        gi = (grp * WINS + win) * (gcw * P) + (tloc * capw + k) * P + p

        idx16 = np.zeros((P, nchunk * 8), np.int16)
        val16 = pair_local.astype(np.int16)
        rows = (gi % 16).astype(np.int64)
        cols = (gi // 16).astype(np.int64)
        for rep in range(8):
            idx16[rep * 16 + rows, cols] = val16

        # fp8 one-hot of local head per slot (pads stay all-zero)
        s8 = np.zeros((P, nchunk * P), dtype=f8np)
        s8[p, jg * P + (h_loc - (tile_id << 7))] = 1.0

        oh = np.zeros((P, (nchunk // 2) * P), dtype=bf16np)
        q_r = 48 * (j & 1) + 24 * half + etype[e_idx]
        col = (jg >> 1) * P + p
        oh[q_r, col] = aug[e_idx].astype(bf16np)

        in_maps.append(
            {
                "emb16": emb16,
                "idx16": idx16,
                "s8": s8,
                "oh": oh,
                "w2": w2,
            }
        )
    return capw, in_maps


def kernel(all_emb, edge_index, edge_type, weight, aug_edge_weight):
    capw, in_maps = _prep(all_emb, edge_index, edge_type, weight,
                          aug_edge_weight)
    if capw not in _NC_CACHE:
        _NC_CACHE[capw] = _build(capw)
    nc = _NC_CACHE[capw]

    trace = bool(int(os.environ.get("KERNEL_TRACE", "0")))
    res = run_bass_kernel_spmd(
        nc,
        in_maps,
        core_ids=list(range(N_CORES)),
        trace=trace,
    )
    kernel._last_results = res
    out = np.concatenate(
        [res.results[c_i]["out"][:NPC] for c_i in range(N_CORES)], axis=0
    )
    return out


# revision 12
# speedup vs baseline: 2.0902x; 1.3862x over previous
"""Trainium2 Bass kernel for nn_Aggregator (GNN relational message passing).

Computes: out[h] = sum_{e: head_e==h} all_emb[tail_e] * weight[type_e] * aug_e

Strategy (8 NeuronCores, SPMD):
  - Shard output nodes (head ranges of 12500) across the 8 cores; each core
    gets exactly the edges whose head falls in its range (host bins them).
    No collective needed - host concatenates the 8 output shards.
  - all_emb is cast to fp16 on host and viewed as 50000 node-PAIR tokens of
    256B each (dma_gather requires 256B-multiple elements).  Pair indices
    fit in 2 int16 windows of 25000, cutting (tile, window) bucket count
    and slot padding (17% vs 34% for fp32 rows).  dma_gather descriptor
    generation on the GpSimd Q7 cores (~8ns/row per queue-pair, 4 pairs
    concurrent when calls rotate queue_num) is the critical path, so each
    group's gather is issued as 4 calls on 4 queues.
  - Host sorts edges by (head tile of 128, tail window); each (tile,window)
    bucket is padded to capw chunks of 128 edges so the schedule is static.
  - Per 128-edge chunk (each edge slot carries a 128-wide token = 2 nodes;
    a host-built half-mask in the weight table selects the right node):
      S    = host-built fp8 one-hot of head (streamed per group, 0/1 exact)
      Wsel = oh^T @ w2 on PE (2 chunks stacked: 96 rows, 256 out cols)
             -> aug_e*weight[type_e] in the edge's half, 0 in the other
      V    = G * wsel on DVE (wsel read from PSUM)                  [e,128]
      acc(psum f32) += S^T @ V on PE (fp8 x fp16, per node tile)
    Tile flush: out[:,c] = acc[:,c] + acc[:,64+c] (ACT stages the high
    half, DVE adds) into a per-group staging buffer; one output DMA per
    7-tile group.
"""

import os

import numpy as np

import concourse.bacc as bacc
import concourse.tile as tile
from concourse import bass, mybir
from concourse.bass_utils import run_bass_kernel_spmd

P = 128
C = 64  # channels
R = 24  # relations
TOK = 128  # fp16 token width (2 nodes x 64 channels)
N_NODES = 100000
NPAIR = N_NODES // 2
N_CORES = 8
NPC = N_NODES // N_CORES  # 12500 nodes per core
TILES = (NPC + P - 1) // P  # 98 output tiles per core
WINS = 2
WSZ = 25000  # window size in pair units (int16 gather index reach)
GROUP_T = 7  # tiles per gather group (98 = 14 * 7)
NGRP = TILES // GROUP_T

_NC_CACHE = {}


def _ap3(ap, offset_elems, mid_step, mid_n, inner_n):
    """[128, mid_n, inner_n] view of a 2D SBUF tile AP with custom strides."""
    return bass.AP(
        ap.tensor,
        ap.offset + offset_elems,
        [list(ap.ap[0]), [mid_step, mid_n], [1, inner_n]],
    )


def _build(capw: int):
    """Build the Bass module for per-(tile,window) chunk capacity capw."""
    cap = WINS * capw  # chunks per tile
    nchunk = TILES * cap
    gc = GROUP_T * cap  # chunks per group
    gcw = GROUP_T * capw  # chunks per (group, window)
    hcw = gcw // 2  # first-half chunks of a (group, window) gather
    npair_oh = cap // 2  # stacked wsel matmuls per tile

    nc = bacc.Bacc("TRN2", target_bir_lowering=False, num_swdge_queues=4)

    f32 = mybir.dt.float32
    f16 = mybir.dt.float16
    bf16 = mybir.dt.bfloat16
    f8 = mybir.dt.float8e4
    i16 = mybir.dt.int16

    emb_d = nc.dram_tensor("emb16", [NPAIR, TOK], f16, kind="ExternalInput")
    idx_d = nc.dram_tensor("idx16", [P, nchunk * 8], i16, kind="ExternalInput")
    s8_d = nc.dram_tensor("s8", [P, nchunk * P], f8, kind="ExternalInput")
    oh_d = nc.dram_tensor("oh", [P, (nchunk // 2) * P], bf16,
                          kind="ExternalInput")
    w2_d = nc.dram_tensor("w2", [P, 2 * TOK], bf16, kind="ExternalInput")
    out_d = nc.dram_tensor("out", [TILES * P, C], f32, kind="ExternalOutput")

    with tile.TileContext(nc) as tc:
        with (
            tc.tile_pool(name="res", bufs=1) as res,
            tc.tile_pool(name="gp", bufs=4) as gp,
            tc.tile_pool(name="ohp", bufs=2) as ohp,
            tc.tile_pool(name="sp", bufs=2) as sp,
            tc.tile_pool(name="vp", bufs=2) as vp,
            tc.tile_pool(name="op", bufs=2) as op,
            tc.tile_pool(name="fp", bufs=2) as fp,
            tc.tile_pool(name="pp", bufs=2, space="PSUM") as pp,
        ):
            idx_t = res.tile([P, nchunk * 8], i16)
            w2_t = res.tile([P, 2 * TOK], bf16)

            nc.sync.dma_start(idx_t[:], idx_d[:])
            nc.sync.dma_start(w2_t[:], w2_d[:])

            ncall = 0
            for g in range(NGRP):
                # gather the whole group; split each window into two calls
                # so all 4 SWDGE queue pairs stay busy
                g_t = gp.tile([P, gc * TOK], f16, tag="g")
                for w in range(WINS):
                    for h0, hn in ((0, hcw), (hcw, gcw - hcw)):
                        nidx = hn * P
                        s0 = (g * WINS + w) * gcw * P + h0 * P
                        nc.gpsimd.dma_gather(
                            out_ap=g_t[
                                :,
                                (w * gcw + h0) * TOK : (w * gcw + h0 + hn)
                                * TOK,
                            ].rearrange("p (j t) -> p j t", t=TOK),
                            in_ap=emb_d[w * WSZ : (w + 1) * WSZ, :],
                            idxs_ap=idx_t[
                                :, s0 // 16 : s0 // 16 + nidx // 16
                            ],
                            num_idxs=nidx,
                            num_idxs_reg=nidx,
                            elem_size=TOK,
                            single_packet=False,
                            queue_num=ncall % 4,
                        )
                        ncall += 1

                # one-hot S and aug*weight one-hot slabs for the group
                s8_t = sp.tile([P, gc * P], f8, tag="s8")
                nc.sync.dma_start(
                    s8_t[:], s8_d[:, g * gc * P : (g + 1) * gc * P]
                )
                oh_t = ohp.tile([P, (gc // 2) * P], bf16, tag="oh")
                nc.sync.dma_start(
                    oh_t[:], oh_d[:, (g * gc // 2) * P : ((g + 1) * gc // 2) * P]
                )

                ostage_t = op.tile([P, GROUP_T * C], f32, tag="ost")

                for tloc in range(GROUP_T):
                    j0 = tloc * cap  # first chunk of the tile within group

                    # one PSUM tile per output tile: wsel (cap*128 fp32)
                    # followed by the 128-col accumulator (fits 4 banks)
                    ps_t = pp.tile([P, cap * TOK + P], f32, tag="ps")

                    # Wsel: one stacked matmul per 2 chunks
                    for b in range(npair_oh):
                        gcol = (j0 // 2 + b) * P
                        nc.tensor.matmul(
                            out=ps_t[:, b * 2 * TOK : (b + 1) * 2 * TOK],
                            lhsT=oh_t[:, gcol : gcol + P],
                            rhs=w2_t[:],
                            start=True,
                            stop=True,
                        )

                    # V = G * wsel for the whole tile in one DVE op
                    # (wsel read straight from PSUM; saves an ACT copy)
                    v_t = vp.tile([P, cap * TOK], f16, tag="v")
                    g_view = _ap3(
                        g_t[:],
                        tloc * capw * TOK,
                        gcw * TOK,
                        WINS,
                        capw * TOK,
                    )
                    nc.vector.tensor_tensor(
                        out=v_t[:].rearrange("p (w x) -> p w x", w=WINS),
                        in0=g_view,
                        in1=ps_t[:, 0 : cap * TOK].rearrange(
                            "p (w x) -> p w x", w=WINS
                        ),
                        op=mybir.AluOpType.mult,
                    )

                    # scatter-accumulate the tile's chunks into PSUM
                    a0 = cap * TOK
                    for q in range(cap):
                        nc.tensor.matmul(
                            out=ps_t[:, a0 : a0 + P],
                            lhsT=s8_t[:, (j0 + q) * P : (j0 + q + 1) * P],
                            rhs=v_t[:, q * TOK : (q + 1) * TOK],
                            start=(q == 0),
                            stop=(q == cap - 1),
                        )

                    # fold the two node-halves and stage (DVE can read at
                    # most one PSUM operand: ACT stages the high half first)
                    fold_t = fp.tile([P, C], f32, tag="fold")
                    nc.scalar.copy(out=fold_t[:], in_=ps_t[:, a0 + C : a0 + 2 * C])
                    nc.vector.tensor_tensor(
                        out=ostage_t[:, tloc * C : (tloc + 1) * C],
                        in0=ps_t[:, a0 : a0 + C],
                        in1=fold_t[:],
                        op=mybir.AluOpType.add,
                    )

                nc.sync.dma_start(
                    out_d[g * GROUP_T * P : (g + 1) * GROUP_T * P, :].rearrange(
                        "(t p) c -> p t c", p=P
                    ),
                    ostage_t[:].rearrange("p (t c) -> p t c", c=C),
                )

    nc.finalize()
    return nc


def _prep(all_emb, edge_index, edge_type, weight, aug_edge_weight):
    """Host-side binning/padding. Returns (capw, in_maps)."""
    import ml_dtypes

    f8np = mybir.dt.np(mybir.dt.float8e4)
    bf16np = ml_dtypes.bfloat16

    head = np.asarray(edge_index[0], dtype=np.int64)
    tail = np.asarray(edge_index[1], dtype=np.int64)
    etype = np.asarray(edge_type, dtype=np.int64)
    aug = np.asarray(aug_edge_weight, dtype=np.float32).reshape(-1)
    emb16 = np.ascontiguousarray(
        np.asarray(all_emb, dtype=np.float16).reshape(NPAIR, TOK)
    )
    w = np.asarray(weight, dtype=np.float32)

    order = np.argsort(head, kind="stable")
    h_s = head[order]
    bounds = np.searchsorted(h_s, np.arange(N_CORES + 1) * NPC)

    capw = 1
    per_core = []
    for c_i in range(N_CORES):
        e_idx = order[bounds[c_i] : bounds[c_i + 1]]
        h_loc = h_s[bounds[c_i] : bounds[c_i + 1]] - c_i * NPC
        t_loc = tail[e_idx]
        win = (t_loc >> 1) // WSZ
        tw = (h_loc >> 7) * WINS + win  # (tile, window) bucket
        cnt = np.bincount(tw, minlength=TILES * WINS)
        capw = max(capw, int(-(-cnt.max() // P)))
        per_core.append((e_idx, h_loc, t_loc, tw, cnt))

    cap = WINS * capw
    nchunk = TILES * cap
    gcw = GROUP_T * capw

    # w2[48s + 24h + r, s*128 + 64h + c] = w[r, c]
    w2 = np.zeros((P, 2 * TOK), dtype=np.float32)
    for s in range(2):
        for h in range(2):
            w2[48 * s + 24 * h : 48 * s + 24 * h + R,
               s * TOK + C * h : s * TOK + C * (h + 1)] = w
    w2 = w2.astype(bf16np)

    in_maps = []
    for c_i in range(N_CORES):
        e_idx, h_loc, t_loc, tw, cnt = per_core[c_i]
        o2 = np.argsort(tw, kind="stable")  # group edges by (tile, window)
        e_idx, h_loc, t_loc, tw = e_idx[o2], h_loc[o2], t_loc[o2], tw[o2]
        starts = np.cumsum(cnt) - cnt
        pos = np.arange(len(e_idx)) - starts[tw]

        tile_id = tw // WINS
        win = tw % WINS
        k = pos >> 7
        p = pos & (P - 1)

        pair_local = (t_loc >> 1) - win * WSZ
        half = t_loc & 1

        # processing chunk id within tile / globally
        j = win * capw + k
        jg = tile_id * cap + j
        # flat gather slot: (group, window) major, then (tloc, k, p)
        grp = tile_id // GROUP_T
        tloc = tile_id % GROUP_T
        gi = (grp * WINS + win) * (gcw * P) + (tloc * capw + k) * P + p

        idx16 = np.zeros((P, nchunk * 8), np.int16)
        val16 = pair_local.astype(np.int16)
        rows = (gi % 16).astype(np.int64)
        cols = (gi // 16).astype(np.int64)
        for rep in range(8):
            idx16[rep * 16 + rows, cols] = val16

        # fp8 one-hot of local head per slot (pads stay all-zero)
        s8 = np.zeros((P, nchunk * P), dtype=f8np)
        s8[p, jg * P + (h_loc - (tile_id << 7))] = 1.0

        oh = np.zeros((P, (nchunk // 2) * P), dtype=bf16np)
        q_r = 48 * (j & 1) + 24 * half + etype[e_idx]
        col = (jg >> 1) * P + p
        oh[q_r, col] = aug[e_idx].astype(bf16np)

        in_maps.append(
            {
                "emb16": emb16,
                "idx16": idx16,
                "s8": s8,
                "oh": oh,
                "w2": w2,
            }
        )
    return capw, in_maps


def kernel(all_emb, edge_index, edge_type, weight, aug_edge_weight):
    capw, in_maps = _prep(all_emb, edge_index, edge_type, weight,
                          aug_edge_weight)
    if capw not in _NC_CACHE:
        _NC_CACHE[capw] = _build(capw)
    nc = _NC_CACHE[capw]

    trace = bool(int(os.environ.get("KERNEL_TRACE", "0")))
    res = run_bass_kernel_spmd(
        nc,
        in_maps,
        core_ids=list(range(N_CORES)),
        trace=trace,
    )
    kernel._last_results = res
    out = np.concatenate(
        [res.results[c_i]["out"][:NPC] for c_i in range(N_CORES)], axis=0
    )
    return out
